# revision 17
# baseline (speedup 1.0000x reference)
"""7x7 valid conv2d on [8192, 8192] fp32, distributed over 8 NeuronCores.

Strategy: column-shard the output across 8 cores (each core computes all 8186
output rows of a 1024-column slice; host-side overlapping column slices
provide the 6-column halo). On each core the convolution runs on the tensor
engine as banded-Toeplitz matmuls: for kernel column j, a stationary matrix
B_j[p, m] = weight[p - m, j] turns a matmul over 128 input rows into a 7-tap
convolution along H producing 122 output rows; the 7 kernel columns
accumulate in PSUM using column-shifted rhs windows. Column sharding makes
the 512-wide PSUM tiles divide the per-core output exactly (2 per row-tile),
so a core runs 68 row-tiles x 2 col-tiles x 7 taps = 952 N=512 matmuls
(vs 1008 for row sharding, where ceil(1024/122) = 9 wastes a partial tile).

Inputs are shipped as fp16 (tolerance 2e-2; fp16 keeps the result ~2e-4),
halving input DMA. Output rows accumulate full-width in SBUF (bias folded
into the PSUM->SBUF copy) and are stored once per row-tile, split 112 rows
on scalar HWDGE + 10 on gpsimd SWDGE: HWDGE distributes a store's row
descriptors across SDMA engines in contiguous blocks of (smallest divisor
of nrows >= nrows/16), so 122 = 2*61 pins to 2 engines (~35 GB/s) while
multiples of 16 spread across all 16 (~218 GB/s). The sync queue carries
only x loads so compute never waits behind store traffic.
"""

import numpy as np

KH = KW = 7
H = W = 8192
OH = H - KH + 1  # 8186
OW = W - KW + 1
NCORES = 8
P = 128
M_FULL = P - (KH - 1)  # 122 output rows per row-tile
N_TILE = 512

CBAND = 1024             # output columns per core (last core: 1018 used)
C_IN = CBAND + KW - 1    # 1030 input columns per core (zero-padded at edge)

MM_DTYPE = "float16"     # matmul operand dtype (1 col/cycle on PE, 2B DMA)
REPS = 1                 # body repetitions (for slope timing only)
X_BUFS = 6               # x row-tile buffers
PS_BUFS = 8
O_BUFS = 4               # output-row-band buffers


def _build_program(bias_val, h_in, cband):
    import concourse.bacc as bacc
    import concourse.mybir as mybir
    import concourse.tile as tile

    mm_dt = getattr(mybir.dt, MM_DTYPE)
    f32 = mybir.dt.float32
    c_in = cband + KW - 1
    oh = h_in - KH + 1

    nc = bacc.Bacc(
        "TRN2",
        target_bir_lowering=False,
        debug=False,
        enable_asserts=False,
        num_devices=NCORES,
    )

    x_dram = nc.dram_tensor("x", [h_in, c_in], mm_dt, kind="ExternalInput")
    wb_dram = nc.dram_tensor("wband", [P, KW, M_FULL], mm_dt, kind="ExternalInput")
    out_dram = nc.dram_tensor("out", [oh, cband], f32, kind="ExternalOutput")

    n_row_tiles = (oh + M_FULL - 1) // M_FULL
    n_col_tiles = (cband + N_TILE - 1) // N_TILE

    with tile.TileContext(nc) as tc:
        with (
            tc.tile_pool(name="const", bufs=1) as cpool,
            tc.tile_pool(name="xp", bufs=X_BUFS) as xpool,
            tc.tile_pool(name="op", bufs=O_BUFS) as opool,
            tc.tile_pool(name="pp", bufs=PS_BUFS, space="PSUM") as pspool,
        ):
            w_sb = cpool.tile([P, KW, M_FULL], mm_dt)
            nc.scalar.dma_start(w_sb[:], wb_dram.ap()[:])

            # PE warm-up: ~10 dummy matmuls on zeros run while the first x
            # load is in flight (the PE is otherwise idle until ~10.5us),
            # tripping the HAM activity monitor so the real matmuls start at
            # 2.4 GHz instead of paying ~3.4us of half-clock cold start.
            warm = cpool.tile([P, N_TILE], mm_dt)
            nc.vector.memset(warm[:], 0.0)
            wps = pspool.tile([M_FULL, N_TILE], f32, tag="ps", name="warm_ps")
            for _ in range(8):
                nc.tensor.matmul(
                    wps[:M_FULL, :], warm[:, :M_FULL], warm[:, :N_TILE],
                    start=True, stop=True,
                )

            def emit_store(r0, m, ow):
                # 10-row remainder stays on scalar too: B(10)=1 spreads over
                # 10 engines, and avoiding gpsimd DMA entirely skips a ~7us
                # SWDGE ring DRAIN in the kernel epilogue.
                s0 = (m // 16) * 16
                if s0 >= 16:
                    nc.scalar.dma_start(
                        out_dram.ap()[r0 : r0 + s0, :], ow[:s0, :cband]
                    )
                if s0 < m:
                    nc.scalar.dma_start(
                        out_dram.ap()[r0 + s0 : r0 + m, :], ow[s0:m, :cband]
                    )

            for _rep in range(REPS):
                for t in range(n_row_tiles):
                    r0 = t * M_FULL
                    m = min(M_FULL, oh - r0)
                    k = m + KH - 1
                    x_sb = xpool.tile([P, c_in], mm_dt, tag="x", name="x_sb")
                    if t == 0 and _rep == 0:
                        # split the first load so col-tile 0's matmuls start
                        # as soon as the first half lands
                        h = N_TILE + KW - 1
                        nc.sync.dma_start(
                            x_sb[:k, :h], x_dram.ap()[r0 : r0 + k, :h]
                        )
                        nc.sync.dma_start(
                            x_sb[:k, h:], x_dram.ap()[r0 : r0 + k, h:]
                        )
                    else:
                        nc.sync.dma_start(x_sb[:k, :], x_dram.ap()[r0 : r0 + k, :])

                    ow = opool.tile([M_FULL, cband], f32, tag="ow", name="ow")
                    for n in range(n_col_tiles):
                        c0 = n * N_TILE
                        wn = min(N_TILE, cband - c0)
                        ps = pspool.tile([M_FULL, N_TILE], f32, tag="ps", name="ps")
                        for j in range(KW):
                            nc.tensor.matmul(
                                ps[:m, :wn], w_sb[:k, j, :m],
                                x_sb[:k, c0 + j : c0 + j + wn],
                                start=(j == 0), stop=(j == KW - 1),
                            )
                        nc.vector.tensor_scalar_add(
                            ow[:m, c0 : c0 + wn], ps[:m, :wn], bias_val
                        )
                    emit_store(r0, m, ow)

    nc.compile()
    return nc


def _make_wband(weight, dtype):
    wband = np.zeros((P, KW, M_FULL), dtype)
    idx = np.arange(M_FULL)
    for j in range(KW):
        for d in range(KH):
            wband[idx + d, j, idx] = weight[d, j].astype(dtype)
    return wband


class Runner:
    """Compiles the per-core program once and exposes repeatable execution
    on all cores via PJRT (the axon path of run_bass_kernel_spmd, inlined so
    inputs can stay device-resident and calls can be timed)."""

    def __init__(self, bias_val, h_in=H, cband=CBAND, n_cores=NCORES):
        self._setup(_build_program(bias_val, h_in, cband), n_cores)

    @classmethod
    def from_nc(cls, nc, n_cores=NCORES):
        r = cls.__new__(cls)
        r._setup(nc, n_cores)
        return r

    def _setup(self, nc, n_cores):
        import jax
        import concourse.mybir as mybir
        from concourse import bass2jax
        from jax.sharding import Mesh, PartitionSpec
        from jax.experimental.shard_map import shard_map

        self.n_cores = n_cores
        self.nc = nc
        bass2jax.install_neuronx_cc_hook()

        partition_name = (
            nc.partition_id_tensor.name if nc.partition_id_tensor else None
        )
        in_names, out_names, out_avals = [], [], []
        for alloc in nc.m.functions[0].allocations:
            if not isinstance(alloc, mybir.MemoryLocationSet):
                continue
            name = alloc.memorylocations[0].name
            if alloc.kind == "ExternalInput":
                if name != partition_name:
                    in_names.append(name)
            elif alloc.kind == "ExternalOutput":
                out_names.append(name)
                out_avals.append(
                    jax.core.ShapedArray(
                        tuple(alloc.tensor_shape), mybir.dt.np(alloc.dtype)
                    )
                )
        self.in_names, self.out_names, self.out_avals = in_names, out_names, out_avals
        n_params = len(in_names)
        donate = tuple(range(n_params, n_params + len(out_names)))

        def _body(*args):
            operands = list(args)
            if nc.partition_id_tensor is not None:
                operands.append(bass2jax.partition_id_tensor())
            outs = bass2jax._bass_exec_p.bind(
                *operands,
                out_avals=tuple(out_avals),
                in_names=tuple(in_names + out_names)
                + ((nc.partition_id_tensor.name,) if nc.partition_id_tensor else ()),
                out_names=tuple(out_names),
                lowering_input_output_aliases=(),
                sim_require_finite=True,
                sim_require_nnan=True,
                nc=nc,
            )
            return tuple(outs)

        devices = jax.devices()[:n_cores]
        self.mesh = Mesh(np.asarray(devices), ("core",))
        self.pspec = PartitionSpec("core")
        in_specs = (self.pspec,) * (n_params + len(out_names))
        out_specs = (self.pspec,) * len(out_names)
        self.fn = jax.jit(
            shard_map(
                _body,
                mesh=self.mesh,
                in_specs=in_specs,
                out_specs=out_specs,
                check_rep=False,
            ),
            donate_argnums=donate,
            keep_unused=True,
        )

    def put_inputs(self, in_maps):
        """device_put per-core input dicts; returns list of jax arrays."""
        import jax
        from jax.sharding import NamedSharding

        sharding = NamedSharding(self.mesh, self.pspec)
        arrs = []
        for name in self.in_names:
            cat = np.concatenate([np.asarray(m[name]) for m in in_maps], axis=0)
            arrs.append(jax.device_put(cat, sharding))
        return arrs

    def zero_outs(self):
        import jax
        from jax.sharding import NamedSharding

        sharding = NamedSharding(self.mesh, self.pspec)
        return tuple(
            jax.device_put(
                np.zeros((self.n_cores * a.shape[0], *a.shape[1:]), a.dtype), sharding
            )
            for a in self.out_avals
        )

    def run(self, in_arrs, out_bufs):
        """One execution; returns new device output arrays (donates out_bufs)."""
        return self.fn(*in_arrs, *out_bufs)

    def gather(self, outs):
        """Device outputs -> list of per-core dicts of np arrays."""
        res = []
        for c in range(self.n_cores):
            d = {}
            for i, name in enumerate(self.out_names):
                a = self.out_avals[i]
                d[name] = np.asarray(outs[i]).reshape(self.n_cores, *a.shape)[c]
            res.append(d)
        return res


def make_in_maps(x, weight, col_starts, c_in=C_IN):
    np_dt = np.dtype(MM_DTYPE)
    x16 = np.asarray(x, dtype=np_dt)
    wband = _make_wband(np.asarray(weight, np.float32), np_dt)
    maps = []
    for c in col_starts:
        sl = x16[:, c : c + c_in]
        if sl.shape[1] < c_in:  # last core: pad unread halo columns
            sl = np.pad(sl, ((0, 0), (0, c_in - sl.shape[1])))
        maps.append({"x": np.ascontiguousarray(sl), "wband": wband})
    return maps


def kernel(x, weight, bias):
    from concourse import bass_utils

    weight = np.asarray(weight, dtype=np.float32)
    bias = np.asarray(bias, dtype=np.float32)

    col_starts = [i * CBAND for i in range(NCORES)]
    nc = _build_program(float(bias[0]), H, CBAND)
    res = bass_utils.run_bass_kernel_spmd(
        nc, make_in_maps(x, weight, col_starts), core_ids=list(range(NCORES))
    )

    out = np.empty((OH, OW), np.float32)
    for c, r in zip(col_starts, res.results):
        w = min(CBAND, OW - c)
        out[:, c : c + w] = r["out"][:, :w]
    return out


# revision 18
# speedup vs baseline: 1.0034x; 1.0034x over previous
"""7x7 valid conv2d on [8192, 8192] fp32, distributed over 8 NeuronCores.

Strategy: column-shard the output across 8 cores (each core computes all 8186
output rows of a 1024-column slice; host-side overlapping column slices
provide the 6-column halo). On each core the convolution runs on the tensor
engine as banded-Toeplitz matmuls: for kernel column j, a stationary matrix
B_j[p, m] = weight[p - m, j] turns a matmul over 128 input rows into a 7-tap
convolution along H producing 122 output rows; the 7 kernel columns
accumulate in PSUM using column-shifted rhs windows. Column sharding makes
the 512-wide PSUM tiles divide the per-core output exactly (2 per row-tile),
so a core runs 68 row-tiles x 2 col-tiles x 7 taps = 952 N=512 matmuls
(vs 1008 for row sharding, where ceil(1024/122) = 9 wastes a partial tile).

Inputs are shipped as fp16 (tolerance 2e-2; fp16 keeps the result ~2e-4),
halving input DMA. Output rows accumulate full-width in SBUF (bias folded
into the PSUM->SBUF copy) and are stored once per row-tile, split 112 rows
on scalar HWDGE + 10 on gpsimd SWDGE: HWDGE distributes a store's row
descriptors across SDMA engines in contiguous blocks of (smallest divisor
of nrows >= nrows/16), so 122 = 2*61 pins to 2 engines (~35 GB/s) while
multiples of 16 spread across all 16 (~218 GB/s). The sync queue carries
only x loads so compute never waits behind store traffic.
"""

import numpy as np

KH = KW = 7
H = W = 8192
OH = H - KH + 1  # 8186
OW = W - KW + 1
NCORES = 8
P = 128
M_FULL = P - (KH - 1)  # 122 output rows per row-tile
N_TILE = 512

CBAND = 1024             # output columns per core (last core: 1018 used)
C_IN = CBAND + KW - 1    # 1030 input columns per core (zero-padded at edge)

MM_DTYPE = "float16"     # matmul operand dtype (1 col/cycle on PE, 2B DMA)
REPS = 1                 # body repetitions (for slope timing only)
X_BUFS = 6               # x row-tile buffers
PS_BUFS = 8
O_BUFS = 4               # output-row-band buffers


def _build_program(bias_val, h_in, cband):
    import concourse.bacc as bacc
    import concourse.mybir as mybir
    import concourse.tile as tile

    mm_dt = getattr(mybir.dt, MM_DTYPE)
    f32 = mybir.dt.float32
    c_in = cband + KW - 1
    oh = h_in - KH + 1

    nc = bacc.Bacc(
        "TRN2",
        target_bir_lowering=False,
        debug=False,
        enable_asserts=False,
        num_devices=NCORES,
    )

    x_dram = nc.dram_tensor("x", [h_in, c_in], mm_dt, kind="ExternalInput")
    wb_dram = nc.dram_tensor("wband", [P, KW, M_FULL], mm_dt, kind="ExternalInput")
    out_dram = nc.dram_tensor("out", [oh, cband], f32, kind="ExternalOutput")

    n_row_tiles = (oh + M_FULL - 1) // M_FULL
    n_col_tiles = (cband + N_TILE - 1) // N_TILE

    with tile.TileContext(nc) as tc:
        with (
            tc.tile_pool(name="const", bufs=1) as cpool,
            tc.tile_pool(name="xp", bufs=X_BUFS) as xpool,
            tc.tile_pool(name="op", bufs=O_BUFS) as opool,
            tc.tile_pool(name="pp", bufs=PS_BUFS, space="PSUM") as pspool,
        ):
            w_sb = cpool.tile([P, KW, M_FULL], mm_dt)
            nc.scalar.dma_start(w_sb[:], wb_dram.ap()[:])

            # PE warm-up: 8 dummy matmuls on zeros run while the first x
            # load is in flight (the PE is otherwise idle until ~9us),
            # tripping the HAM activity monitor so the real matmuls start at
            # 2.4 GHz instead of paying ~3.4us of half-clock cold start.
            warm = cpool.tile([P, N_TILE], mm_dt)
            nc.vector.memset(warm[:], 0.0)
            wps = pspool.tile([M_FULL, N_TILE], f32, tag="ps", name="warm_ps")
            for _ in range(8):
                nc.tensor.matmul(
                    wps[:M_FULL, :], warm[:, :M_FULL], warm[:, :N_TILE],
                    start=True, stop=True,
                )

            def emit_store(r0, m, ow):
                # 10-row remainder stays on scalar too: B(10)=1 spreads over
                # 10 engines, and avoiding gpsimd DMA entirely skips a ~7us
                # SWDGE ring DRAIN in the kernel epilogue.
                s0 = (m // 16) * 16
                if s0 >= 16:
                    nc.scalar.dma_start(
                        out_dram.ap()[r0 : r0 + s0, :], ow[:s0, :cband]
                    )
                if s0 < m:
                    nc.scalar.dma_start(
                        out_dram.ap()[r0 + s0 : r0 + m, :], ow[s0:m, :cband]
                    )

            for _rep in range(REPS):
                for t in range(n_row_tiles):
                    r0 = t * M_FULL
                    m = min(M_FULL, oh - r0)
                    k = m + KH - 1
                    x_sb = xpool.tile([P, c_in], mm_dt, tag="x", name="x_sb")
                    if t == 0 and _rep == 0:
                        # split the first load so col-tile 0's matmuls start
                        # as soon as the first half lands
                        h = N_TILE + KW - 1
                        nc.sync.dma_start(
                            x_sb[:k, :h], x_dram.ap()[r0 : r0 + k, :h]
                        )
                        nc.sync.dma_start(
                            x_sb[:k, h:], x_dram.ap()[r0 : r0 + k, h:]
                        )
                    else:
                        nc.sync.dma_start(x_sb[:k, :], x_dram.ap()[r0 : r0 + k, :])

                    ow = opool.tile([M_FULL, cband], f32, tag="ow", name="ow")
                    for n in range(n_col_tiles):
                        c0 = n * N_TILE
                        wn = min(N_TILE, cband - c0)
                        ps = pspool.tile([M_FULL, N_TILE], f32, tag="ps", name="ps")
                        for j in range(KW):
                            nc.tensor.matmul(
                                ps[:m, :wn], w_sb[:k, j, :m],
                                x_sb[:k, c0 + j : c0 + j + wn],
                                start=(j == 0), stop=(j == KW - 1),
                            )
                        nc.vector.tensor_scalar_add(
                            ow[:m, c0 : c0 + wn], ps[:m, :wn], bias_val
                        )
                    emit_store(r0, m, ow)

    nc.compile()
    return nc


def _make_wband(weight, dtype):
    wband = np.zeros((P, KW, M_FULL), dtype)
    idx = np.arange(M_FULL)
    for j in range(KW):
        for d in range(KH):
            wband[idx + d, j, idx] = weight[d, j].astype(dtype)
    return wband


class Runner:
    """Compiles the per-core program once and exposes repeatable execution
    on all cores via PJRT (the axon path of run_bass_kernel_spmd, inlined so
    inputs can stay device-resident and calls can be timed)."""

    def __init__(self, bias_val, h_in=H, cband=CBAND, n_cores=NCORES):
        self._setup(_build_program(bias_val, h_in, cband), n_cores)

    @classmethod
    def from_nc(cls, nc, n_cores=NCORES):
        r = cls.__new__(cls)
        r._setup(nc, n_cores)
        return r

    def _setup(self, nc, n_cores):
        import jax
        import concourse.mybir as mybir
        from concourse import bass2jax
        from jax.sharding import Mesh, PartitionSpec
        from jax.experimental.shard_map import shard_map

        self.n_cores = n_cores
        self.nc = nc
        bass2jax.install_neuronx_cc_hook()

        partition_name = (
            nc.partition_id_tensor.name if nc.partition_id_tensor else None
        )
        in_names, out_names, out_avals = [], [], []
        for alloc in nc.m.functions[0].allocations:
            if not isinstance(alloc, mybir.MemoryLocationSet):
                continue
            name = alloc.memorylocations[0].name
            if alloc.kind == "ExternalInput":
                if name != partition_name:
                    in_names.append(name)
            elif alloc.kind == "ExternalOutput":
                out_names.append(name)
                out_avals.append(
                    jax.core.ShapedArray(
                        tuple(alloc.tensor_shape), mybir.dt.np(alloc.dtype)
                    )
                )
        self.in_names, self.out_names, self.out_avals = in_names, out_names, out_avals
        n_params = len(in_names)
        donate = tuple(range(n_params, n_params + len(out_names)))

        def _body(*args):
            operands = list(args)
            if nc.partition_id_tensor is not None:
                operands.append(bass2jax.partition_id_tensor())
            outs = bass2jax._bass_exec_p.bind(
                *operands,
                out_avals=tuple(out_avals),
                in_names=tuple(in_names + out_names)
                + ((nc.partition_id_tensor.name,) if nc.partition_id_tensor else ()),
                out_names=tuple(out_names),
                lowering_input_output_aliases=(),
                sim_require_finite=True,
                sim_require_nnan=True,
                nc=nc,
            )
            return tuple(outs)

        devices = jax.devices()[:n_cores]
        self.mesh = Mesh(np.asarray(devices), ("core",))
        self.pspec = PartitionSpec("core")
        in_specs = (self.pspec,) * (n_params + len(out_names))
        out_specs = (self.pspec,) * len(out_names)
        self.fn = jax.jit(
            shard_map(
                _body,
                mesh=self.mesh,
                in_specs=in_specs,
                out_specs=out_specs,
                check_rep=False,
            ),
            donate_argnums=donate,
            keep_unused=True,
        )

    def put_inputs(self, in_maps):
        """device_put per-core input dicts; returns list of jax arrays."""
        import jax
        from jax.sharding import NamedSharding

        sharding = NamedSharding(self.mesh, self.pspec)
        arrs = []
        for name in self.in_names:
            cat = np.concatenate([np.asarray(m[name]) for m in in_maps], axis=0)
            arrs.append(jax.device_put(cat, sharding))
        return arrs

    def zero_outs(self):
        import jax
        from jax.sharding import NamedSharding

        sharding = NamedSharding(self.mesh, self.pspec)
        return tuple(
            jax.device_put(
                np.zeros((self.n_cores * a.shape[0], *a.shape[1:]), a.dtype), sharding
            )
            for a in self.out_avals
        )

    def run(self, in_arrs, out_bufs):
        """One execution; returns new device output arrays (donates out_bufs)."""
        return self.fn(*in_arrs, *out_bufs)

    def gather(self, outs):
        """Device outputs -> list of per-core dicts of np arrays."""
        res = []
        for c in range(self.n_cores):
            d = {}
            for i, name in enumerate(self.out_names):
                a = self.out_avals[i]
                d[name] = np.asarray(outs[i]).reshape(self.n_cores, *a.shape)[c]
            res.append(d)
        return res


def make_in_maps(x, weight, col_starts, c_in=C_IN):
    np_dt = np.dtype(MM_DTYPE)
    x16 = np.asarray(x, dtype=np_dt)
    wband = _make_wband(np.asarray(weight, np.float32), np_dt)
    maps = []
    for c in col_starts:
        sl = x16[:, c : c + c_in]
        if sl.shape[1] < c_in:  # last core: pad unread halo columns
            sl = np.pad(sl, ((0, 0), (0, c_in - sl.shape[1])))
        maps.append({"x": np.ascontiguousarray(sl), "wband": wband})
    return maps


def kernel(x, weight, bias):
    from concourse import bass_utils

    weight = np.asarray(weight, dtype=np.float32)
    bias = np.asarray(bias, dtype=np.float32)

    col_starts = [i * CBAND for i in range(NCORES)]
    nc = _build_program(float(bias[0]), H, CBAND)
    res = bass_utils.run_bass_kernel_spmd(
        nc, make_in_maps(x, weight, col_starts), core_ids=list(range(NCORES))
    )

    out = np.empty((OH, OW), np.float32)
    for c, r in zip(col_starts, res.results):
        w = min(CBAND, OW - c)
        out[:, c : c + w] = r["out"][:, :w]
    return out


# revision 21
# speedup vs baseline: 1.0051x; 1.0017x over previous
"""7x7 valid conv2d on [8192, 8192] fp32, distributed over 8 NeuronCores.

Strategy: column-shard the output across 8 cores (each core computes all 8186
output rows of a 1024-column slice; host-side overlapping column slices
provide the 6-column halo). On each core the convolution runs on the tensor
engine as banded-Toeplitz matmuls: for kernel column j, a stationary matrix
B_j[p, m] = weight[p - m, j] turns a matmul over 128 input rows into a 7-tap
convolution along H producing 122 output rows; the 7 kernel columns
accumulate in PSUM using column-shifted rhs windows. Column sharding makes
the 512-wide PSUM tiles divide the per-core output exactly (2 per row-tile),
so a core runs 68 row-tiles x 2 col-tiles x 7 taps = 952 N=512 matmuls
(vs 1008 for row sharding, where ceil(1024/122) = 9 wastes a partial tile).

Inputs are shipped as fp16 (tolerance 2e-2; fp16 keeps the result ~2e-4),
halving input DMA. Output rows accumulate full-width in SBUF (bias folded
into the PSUM->SBUF copy) and are stored once per row-tile, split 112 rows
on scalar HWDGE + 10 on gpsimd SWDGE: HWDGE distributes a store's row
descriptors across SDMA engines in contiguous blocks of (smallest divisor
of nrows >= nrows/16), so 122 = 2*61 pins to 2 engines (~35 GB/s) while
multiples of 16 spread across all 16 (~218 GB/s). The sync queue carries
only x loads so compute never waits behind store traffic.
"""

import numpy as np

KH = KW = 7
H = W = 8192
OH = H - KH + 1  # 8186
OW = W - KW + 1
NCORES = 8
P = 128
M_FULL = P - (KH - 1)  # 122 output rows per row-tile
N_TILE = 512

CBAND = 1024             # output columns per core (last core: 1018 used)
C_IN = CBAND + KW - 1    # 1030 input columns per core (zero-padded at edge)

MM_DTYPE = "float16"     # matmul operand dtype (1 col/cycle on PE, 2B DMA)
REPS = 1                 # body repetitions (for slope timing only)
X_BUFS = 6               # x row-tile buffers
PS_BUFS = 8
O_BUFS = 4               # output-row-band buffers


def _build_program(bias_val, h_in, cband):
    import concourse.bacc as bacc
    import concourse.mybir as mybir
    import concourse.tile as tile

    mm_dt = getattr(mybir.dt, MM_DTYPE)
    f32 = mybir.dt.float32
    c_in = cband + KW - 1
    oh = h_in - KH + 1

    nc = bacc.Bacc(
        "TRN2",
        target_bir_lowering=False,
        debug=False,
        enable_asserts=False,
        num_devices=NCORES,
    )

    x_dram = nc.dram_tensor("x", [h_in, c_in], mm_dt, kind="ExternalInput")
    wb_dram = nc.dram_tensor("wband", [P, KW, M_FULL], mm_dt, kind="ExternalInput")
    out_dram = nc.dram_tensor("out", [oh, cband], f32, kind="ExternalOutput")

    n_row_tiles = (oh + M_FULL - 1) // M_FULL
    n_col_tiles = (cband + N_TILE - 1) // N_TILE

    with tile.TileContext(nc) as tc:
        with (
            tc.tile_pool(name="const", bufs=1) as cpool,
            tc.tile_pool(name="xp", bufs=X_BUFS) as xpool,
            tc.tile_pool(name="op", bufs=O_BUFS) as opool,
            tc.tile_pool(name="pp", bufs=PS_BUFS, space="PSUM") as pspool,
        ):
            w_sb = cpool.tile([P, KW, M_FULL], mm_dt)
            nc.scalar.dma_start(w_sb[:], wb_dram.ap()[:])

            # PE warm-up: 8 dummy matmuls on zeros run while the first x
            # load is in flight (the PE is otherwise idle until ~9us),
            # tripping the HAM activity monitor so the real matmuls start at
            # 2.4 GHz instead of paying ~3.4us of half-clock cold start.
            warm = cpool.tile([P, N_TILE], mm_dt)
            nc.vector.memset(warm[:], 0.0)
            wps = pspool.tile([M_FULL, N_TILE], f32, tag="ps", name="warm_ps")
            for _ in range(8):
                nc.tensor.matmul(
                    wps[:M_FULL, :], warm[:, :M_FULL], warm[:, :N_TILE],
                    start=True, stop=True,
                )

            def emit_store(r0, m, ow):
                # 10-row remainder stays on scalar too: B(10)=1 spreads over
                # 10 engines, and avoiding gpsimd DMA entirely skips a ~7us
                # SWDGE ring DRAIN in the kernel epilogue.
                s0 = (m // 16) * 16
                if s0 >= 16:
                    nc.scalar.dma_start(
                        out_dram.ap()[r0 : r0 + s0, :], ow[:s0, :cband]
                    )
                if s0 < m:
                    nc.scalar.dma_start(
                        out_dram.ap()[r0 + s0 : r0 + m, :], ow[s0:m, :cband]
                    )

            for _rep in range(REPS):
                for t in range(n_row_tiles):
                    r0 = t * M_FULL
                    m = min(M_FULL, oh - r0)
                    k = m + KH - 1
                    x_sb = xpool.tile([P, c_in], mm_dt, tag="x", name="x_sb")
                    if t == 0 and _rep == 0:
                        # split the first load so col-tile 0's matmuls start
                        # as soon as the first half lands
                        h = N_TILE + KW - 1
                        nc.sync.dma_start(
                            x_sb[:k, :h], x_dram.ap()[r0 : r0 + k, :h]
                        )
                        nc.sync.dma_start(
                            x_sb[:k, h:], x_dram.ap()[r0 : r0 + k, h:]
                        )
                    else:
                        nc.sync.dma_start(x_sb[:k, :], x_dram.ap()[r0 : r0 + k, :])

                    ow = opool.tile([M_FULL, cband], f32, tag="ow", name="ow")
                    for n in range(n_col_tiles):
                        c0 = n * N_TILE
                        wn = min(N_TILE, cband - c0)
                        ps = pspool.tile([M_FULL, N_TILE], f32, tag="ps", name="ps")
                        for j in range(KW):
                            nc.tensor.matmul(
                                ps[:m, :wn], w_sb[:k, j, :m],
                                x_sb[:k, c0 + j : c0 + j + wn],
                                start=(j == 0), stop=(j == KW - 1),
                            )
                        nc.vector.tensor_scalar_add(
                            ow[:m, c0 : c0 + wn], ps[:m, :wn], bias_val
                        )
                    emit_store(r0, m, ow)

    nc.compile()
    return nc


def _make_wband(weight, dtype):
    wband = np.zeros((P, KW, M_FULL), dtype)
    idx = np.arange(M_FULL)
    for j in range(KW):
        for d in range(KH):
            wband[idx + d, j, idx] = weight[d, j].astype(dtype)
    return wband


class Runner:
    """Compiles the per-core program once and exposes repeatable execution
    on all cores via PJRT (the axon path of run_bass_kernel_spmd, inlined so
    inputs can stay device-resident and calls can be timed)."""

    def __init__(self, bias_val, h_in=H, cband=CBAND, n_cores=NCORES):
        self._setup(_build_program(bias_val, h_in, cband), n_cores)

    @classmethod
    def from_nc(cls, nc, n_cores=NCORES):
        r = cls.__new__(cls)
        r._setup(nc, n_cores)
        return r

    def _setup(self, nc, n_cores):
        import jax
        import concourse.mybir as mybir
        from concourse import bass2jax
        from jax.sharding import Mesh, PartitionSpec
        from jax.experimental.shard_map import shard_map

        self.n_cores = n_cores
        self.nc = nc
        bass2jax.install_neuronx_cc_hook()

        partition_name = (
            nc.partition_id_tensor.name if nc.partition_id_tensor else None
        )
        in_names, out_names, out_avals = [], [], []
        for alloc in nc.m.functions[0].allocations:
            if not isinstance(alloc, mybir.MemoryLocationSet):
                continue
            name = alloc.memorylocations[0].name
            if alloc.kind == "ExternalInput":
                if name != partition_name:
                    in_names.append(name)
            elif alloc.kind == "ExternalOutput":
                out_names.append(name)
                out_avals.append(
                    jax.core.ShapedArray(
                        tuple(alloc.tensor_shape), mybir.dt.np(alloc.dtype)
                    )
                )
        self.in_names, self.out_names, self.out_avals = in_names, out_names, out_avals
        n_params = len(in_names)
        donate = tuple(range(n_params, n_params + len(out_names)))

        def _body(*args):
            operands = list(args)
            if nc.partition_id_tensor is not None:
                operands.append(bass2jax.partition_id_tensor())
            outs = bass2jax._bass_exec_p.bind(
                *operands,
                out_avals=tuple(out_avals),
                in_names=tuple(in_names + out_names)
                + ((nc.partition_id_tensor.name,) if nc.partition_id_tensor else ()),
                out_names=tuple(out_names),
                lowering_input_output_aliases=(),
                sim_require_finite=True,
                sim_require_nnan=True,
                nc=nc,
            )
            return tuple(outs)

        devices = jax.devices()[:n_cores]
        self.mesh = Mesh(np.asarray(devices), ("core",))
        self.pspec = PartitionSpec("core")
        in_specs = (self.pspec,) * (n_params + len(out_names))
        out_specs = (self.pspec,) * len(out_names)
        self.fn = jax.jit(
            shard_map(
                _body,
                mesh=self.mesh,
                in_specs=in_specs,
                out_specs=out_specs,
                check_rep=False,
            ),
            donate_argnums=donate,
            keep_unused=True,
        )

    def put_inputs(self, in_maps):
        """device_put per-core input dicts; returns list of jax arrays."""
        import jax
        from jax.sharding import NamedSharding

        sharding = NamedSharding(self.mesh, self.pspec)
        arrs = []
        for name in self.in_names:
            cat = np.concatenate([np.asarray(m[name]) for m in in_maps], axis=0)
            arrs.append(jax.device_put(cat, sharding))
        return arrs

    def zero_outs(self):
        import jax
        from jax.sharding import NamedSharding

        sharding = NamedSharding(self.mesh, self.pspec)
        return tuple(
            jax.device_put(
                np.zeros((self.n_cores * a.shape[0], *a.shape[1:]), a.dtype), sharding
            )
            for a in self.out_avals
        )

    def run(self, in_arrs, out_bufs):
        """One execution; returns new device output arrays (donates out_bufs)."""
        return self.fn(*in_arrs, *out_bufs)

    def gather(self, outs):
        """Device outputs -> list of per-core dicts of np arrays."""
        res = []
        for c in range(self.n_cores):
            d = {}
            for i, name in enumerate(self.out_names):
                a = self.out_avals[i]
                d[name] = np.asarray(outs[i]).reshape(self.n_cores, *a.shape)[c]
            res.append(d)
        return res


def make_in_maps(x, weight, col_starts, c_in=C_IN):
    np_dt = np.dtype(MM_DTYPE)
    x16 = np.asarray(x, dtype=np_dt)
    wband = _make_wband(np.asarray(weight, np.float32), np_dt)
    maps = []
    for c in col_starts:
        sl = x16[:, c : c + c_in]
        if sl.shape[1] < c_in:  # last core: pad unread halo columns
            sl = np.pad(sl, ((0, 0), (0, c_in - sl.shape[1])))
        maps.append({"x": np.ascontiguousarray(sl), "wband": wband})
    return maps


def kernel(x, weight, bias):
    from concourse import bass_utils

    weight = np.asarray(weight, dtype=np.float32)
    bias = np.asarray(bias, dtype=np.float32)

    col_starts = [i * CBAND for i in range(NCORES)]
    nc = _build_program(float(bias[0]), H, CBAND)
    res = bass_utils.run_bass_kernel_spmd(
        nc, make_in_maps(x, weight, col_starts), core_ids=list(range(NCORES))
    )

    out = np.empty((OH, OW), np.float32)
    for c, r in zip(col_starts, res.results):
        w = min(CBAND, OW - c)
        out[:, c : c + w] = r["out"][:, :w]
    return out
